# revision 4
# baseline (speedup 1.0000x reference)
"""Trainium2 Bass kernel for nn_LstmModel2 (LSTM over stocks + NeuralSort + MLP).

Sharding: data-parallel over batch B=64 across 8 cores (8 batches/core).
Each core runs the LSTM over 800 independent sequences (8 batches x 100 stocks)
in feature-on-partition layout (H=120 partitions, batch on the free axis),
with the full NeuralSort + MLP tail computed locally per core.
"""

import numpy as np

for _p in ("/root/.axon_site/_ro/trn_rl_repo", "/root/.axon_site/_ro/pypackages",
           "/opt/trn_rl_repo", "/opt/pypackages"):
    import sys
    if _p not in sys.path:
        sys.path.append(_p)

import concourse.bass as bass
import concourse.bacc as bacc
import concourse.tile as tile
from concourse import mybir
from concourse import bass_utils
from concourse._compat import with_exitstack

F32 = mybir.dt.float32
AF = mybir.ActivationFunctionType
ALU = mybir.AluOpType

B, T, S, A, H = 64, 128, 100, 16, 120
TAU = 5.0
NCORES = 8
BL = B // NCORES          # batches per core
NL = BL * S               # sequences per core (free axis total)
NW = NL // 2              # per-wave free size (two pipelined waves)
G4 = 4 * H                # 480

# gate reorder: torch rows [i f g o] -> [i f o g] so i,f,o are contiguous
_PERM = np.concatenate([np.arange(0, 120), np.arange(120, 240),
                        np.arange(360, 480), np.arange(240, 360)])


@with_exitstack
def _build(ctx, tc, aps, b_lin_f):
    nc = tc.nc
    xT, wx, wh, wlin, scal, ones100, ones100c, ones8, b2r, b3r, w2, w3, ident, out = (
        aps["xT"], aps["wx"], aps["wh"], aps["wlin"], aps["scaling"],
        aps["ones100"], aps["ones100c"], aps["ones8"], aps["b2r"], aps["b3r"],
        aps["w2"], aps["w3"], aps["identity"], aps["out"])

    consts = ctx.enter_context(tc.tile_pool(name="consts", bufs=1))
    wx_t = consts.tile_from(wx)          # (17, 480)
    wh_t = consts.tile_from(wh)          # (120, 480)
    wlin_t = consts.tile_from(wlin)      # (120, 1)
    scal_t = consts.tile_from(scal)      # (1, 100)
    on100_t = consts.tile_from(ones100)  # (1, 100)
    on100c_t = consts.tile_from(ones100c)  # (100, 1)
    on8_t = consts.tile_from(ones8)      # (1, 8)
    b2r_t = consts.tile_from(b2r)        # (1, 256)
    b3r_t = consts.tile_from(b3r)        # (1, 100)
    w2_t = consts.tile_from(w2)          # (100, 25600)
    w3_t = consts.tile_from(w3)          # (128, 200)
    id_t = consts.tile_from(ident)      # (128, 128)

    xpool = ctx.enter_context(tc.tile_pool(name="xin", bufs=4))
    sb = ctx.enter_context(tc.tile_pool(name="work", bufs=3))
    state = ctx.enter_context(tc.tile_pool(name="state", bufs=2))

    h_prev = [None, None]
    c_prev = [None, None]

    with tc.tile_pool(name="gps", bufs=2, space="PSUM") as gps:
        for t in range(T):
            xt = xpool.tile([17, NL], F32, tag="xt")
            nc.sync.dma_start(xt[:], xT[t])
            for w in range(2):
                ps = gps.tile([H, 2048], F32, tag="ps")
                xw = xt[:, NW * w:NW * (w + 1)]
                for g in range(4):
                    nc.tensor.matmul(ps[:, 512 * g:512 * g + NW],
                                     wx_t[:, H * g:H * (g + 1)], xw,
                                     start=True, stop=(t == 0))
                if t > 0:
                    for g in range(4):
                        nc.tensor.matmul(ps[:, 512 * g:512 * g + NW],
                                         wh_t[:, H * g:H * (g + 1)],
                                         h_prev[w][:], start=False, stop=True)
                # i,f,o sigmoid fused (3 x NW strided in psum -> dense sbuf)
                ifo = sb.tile([H, 3 * NW], F32, tag="ifo")
                ps3 = ps[:].rearrange("p (g c) -> p g c", g=4)
                nc.scalar.activation(
                    ifo[:].rearrange("p (g c) -> p g c", g=3),
                    ps3[:, 0:3, 0:NW], AF.Sigmoid)
                gt = sb.tile([H, NW], F32, tag="gt")
                nc.scalar.activation(gt[:], ps[:, 1536:1536 + NW], AF.Tanh)

                i_ap = ifo[:, 0:NW]
                f_ap = ifo[:, NW:2 * NW]
                o_ap = ifo[:, 2 * NW:3 * NW]
                c_new = state.tile([H, NW], F32, tag=f"c{w}")
                if t == 0:
                    nc.vector.tensor_mul(c_new[:], i_ap, gt[:])
                else:
                    u = sb.tile([H, NW], F32, tag="u")
                    nc.vector.tensor_mul(u[:], i_ap, gt[:])
                    fc = sb.tile([H, NW], F32, tag="fc")
                    nc.vector.tensor_mul(fc[:], f_ap, c_prev[w][:])
                    nc.vector.tensor_add(c_new[:], fc[:], u[:])
                tch = sb.tile([H, NW], F32, tag="tch")
                nc.scalar.activation(tch[:], c_new[:], AF.Tanh)
                h_new = state.tile([H, NW], F32, tag=f"h{w}")
                nc.vector.tensor_mul(h_new[:], o_ap, tch[:])
                h_prev[w], c_prev[w] = h_new, c_new

    # ---------------- tail: score + NeuralSort + MLP ----------------
    tsb = ctx.enter_context(tc.tile_pool(name="tailsb", bufs=1))
    with tc.tile_pool(name="tailps1", bufs=1, space="PSUM") as tp1:
        # s = h_last @ w_lin + b_lin  -> (1, 800)
        s_ps = tp1.tile([1, 1024], F32, tag="rowps")
        for w in range(2):
            nc.tensor.matmul(s_ps[:, 512 * w:512 * w + NW], wlin_t[:],
                             h_prev[w][:], start=True, stop=True)
        s_row = tsb.tile([1, NL], F32, tag="srow")
        negs = tsb.tile([1, NL], F32, tag="negs")
        for w in range(2):
            nc.vector.tensor_scalar_add(s_row[:, NW * w:NW * (w + 1)],
                                        s_ps[:, 512 * w:512 * w + NW], b_lin_f)
        nc.vector.tensor_scalar_mul(negs[:], s_row[:], -1.0)

        # D[i, b*?+j] = s_b[i] - s_b[j], per-b blocks at 256-col slots
        d_ps = tp1.tile([S, 2048], F32, tag="dps")
        for b in range(BL):
            sb_sl = s_row[:, S * b:S * (b + 1)]
            nc.tensor.matmul(d_ps[:, 256 * b:256 * b + S], sb_sl, on100_t[:],
                             start=True, stop=False)
            nc.tensor.matmul(d_ps[:, 256 * b:256 * b + S], on100_t[:],
                             negs[:, S * b:S * (b + 1)], start=False, stop=True)
        a_s = tsb.tile([S, NL], F32, tag="a_s")
        nc.scalar.activation(
            a_s[:].rearrange("p (b c) -> p b c", b=BL),
            d_ps[:].rearrange("p (b c) -> p b c", b=BL)[:, :, 0:S], AF.Abs)

        # rowsum (as a row) = ones^T @ A_s ; negated to sbuf
        rs_ps = tp1.tile([1, 1024], F32, tag="rowps")
        for w in range(2):
            nc.tensor.matmul(rs_ps[:, 512 * w:512 * w + NW], on100c_t[:],
                             a_s[:, NW * w:NW * (w + 1)], start=True, stop=True)
        nrs = tsb.tile([1, NL], F32, tag="nrs")
        for w in range(2):
            nc.vector.tensor_scalar_mul(nrs[:, NW * w:NW * (w + 1)],
                                        rs_ps[:, 512 * w:512 * w + NW], -1.0)

        # Mt[i, (b,j)] = s_b[i]*scaling[j] - rowsum_b[i];  E2 = exp(Mt/TAU)
        e_ps = tp1.tile([S, 2048], F32, tag="dps")
        for b in range(BL):
            nc.tensor.matmul(e_ps[:, 256 * b:256 * b + S],
                             s_row[:, S * b:S * (b + 1)], scal_t[:],
                             start=True, stop=False)
            nc.tensor.matmul(e_ps[:, 256 * b:256 * b + S],
                             nrs[:, S * b:S * (b + 1)], on100_t[:],
                             start=False, stop=True)
        e2 = tsb.tile([S, NL], F32, tag="e2")
        nc.scalar.activation(
            e2[:].rearrange("p (b c) -> p b c", b=BL),
            e_ps[:].rearrange("p (b c) -> p b c", b=BL)[:, :, 0:S],
            AF.Exp, scale=1.0 / TAU)

        # denominator over partitions via ones-matmul; reciprocal; broadcast
        den_ps = tp1.tile([1, 1024], F32, tag="rowps")
        for w in range(2):
            nc.tensor.matmul(den_ps[:, 512 * w:512 * w + NW], on100c_t[:],
                             e2[:, NW * w:NW * (w + 1)], start=True, stop=True)
        rden = tsb.tile([1, NL], F32, tag="rden")
        for w in range(2):
            nc.vector.reciprocal(rden[:, NW * w:NW * (w + 1)],
                                 den_ps[:, 512 * w:512 * w + NW])
        rd_ps = tp1.tile([S, 1024], F32, tag="rdps")
        for w in range(2):
            nc.tensor.matmul(rd_ps[:, 512 * w:512 * w + NW], on100_t[:],
                             rden[:, NW * w:NW * (w + 1)], start=True, stop=True)
        e2n = tsb.tile([S, NL], F32, tag="e2n")
        nc.vector.tensor_mul(
            e2n[:].rearrange("p (w c) -> p w c", w=2),
            e2[:].rearrange("p (w c) -> p w c", w=2),
            rd_ps[:].rearrange("p (w c) -> p w c", w=2)[:, :, 0:NW])

    with tc.tile_pool(name="tailps2", bufs=1, space="PSUM") as tp2:
        # MLP layer 1: z[b, k] accumulated over j-chunks of the contraction
        z_ps = tp2.tile([BL, 256], F32, tag="zps")
        e2n3 = e2n[:].rearrange("p (b c) -> p b c", b=BL)
        for j in range(S):
            nc.tensor.matmul(z_ps[:], e2n3[:, :, j], w2_t[:, 256 * j:256 * (j + 1)],
                             start=(j == 0), stop=False)
        nc.tensor.matmul(z_ps[:], on8_t[:], b2r_t[:], start=False, stop=True)
        zr = tsb.tile([BL, 256], F32, tag="zr")
        nc.scalar.activation(zr[:], z_ps[:], AF.Relu)

        # transpose z (8,256) -> (256, 8) in two 128-chunks
        zT_ps = tp2.tile([128, 16], F32, tag="zT")
        for ch in range(2):
            nc.tensor.transpose(zT_ps[:, 8 * ch:8 * ch + 8],
                                zr[:, 128 * ch:128 * (ch + 1)],
                                id_t[0:BL, 0:BL])
        zT_sb = tsb.tile([128, 16], F32, tag="zTsb")
        nc.vector.tensor_copy(zT_sb[:], zT_ps[:])

        # layer 2: z3 (8, 100)
        z3_ps = tp2.tile([BL, 128], F32, tag="z3")
        nc.tensor.matmul(z3_ps[:, 0:S], zT_sb[:, 0:8], w3_t[:, 0:100],
                         start=True, stop=False)
        nc.tensor.matmul(z3_ps[:, 0:S], zT_sb[:, 8:16], w3_t[:, 100:200],
                         start=False, stop=False)
        nc.tensor.matmul(z3_ps[:, 0:S], on8_t[:], b3r_t[:],
                         start=False, stop=True)

        # final softmax along free dim
        nmx = tsb.tile([BL, 1], F32, tag="nmx")
        nc.vector.tensor_reduce(nmx[:], z3_ps[:, 0:S], axis=mybir.AxisListType.X,
                                op=ALU.max, negate=True)
        e3 = tsb.tile([BL, S], F32, tag="e3")
        den3 = tsb.tile([BL, 1], F32, tag="den3")
        nc.scalar.activation(e3[:], z3_ps[:, 0:S], AF.Exp, bias=nmx[:],
                             accum_out=den3[:])
        rd3 = tsb.tile([BL, 1], F32, tag="rd3")
        nc.vector.reciprocal(rd3[:], den3[:])
        outt = tsb.tile([BL, S], F32, tag="outt")
        nc.vector.tensor_scalar_mul(outt[:], e3[:], rd3[:])
        nc.sync.dma_start(out[:], outt[:])


def _prep_shared(W_ih, W_hh, b_ih, b_hh, w_lin, W2, b2, W3, b3):
    wx = np.vstack([W_ih.T, (b_ih + b_hh)[None, :]])[:, _PERM]      # (17, 480)
    wh = np.ascontiguousarray(W_hh.T[:, _PERM])                      # (120, 480)
    wlin = np.ascontiguousarray(w_lin.T)                             # (120, 1)
    scal = (S + 1 - 2 * np.arange(1, S + 1, dtype=np.float32))[None, :]
    w2 = np.ascontiguousarray(
        W2.reshape(256, S, S).transpose(2, 1, 0).reshape(S, S * 256))  # (100, 25600)
    w3 = np.ascontiguousarray(
        W3.T.reshape(2, 128, S).transpose(1, 0, 2).reshape(128, 2 * S))  # (128, 200)
    return {
        "wx": wx.astype(np.float32),
        "wh": wh.astype(np.float32),
        "wlin": wlin.astype(np.float32),
        "scaling": scal.astype(np.float32),
        "ones100": np.ones((1, S), np.float32),
        "ones100c": np.ones((S, 1), np.float32),
        "ones8": np.ones((1, BL), np.float32),
        "b2r": b2[None, :].astype(np.float32),
        "b3r": b3[None, :].astype(np.float32),
        "w2": w2.astype(np.float32),
        "w3": w3.astype(np.float32),
        "identity": np.eye(128, dtype=np.float32),
    }


_RESULT_CACHE = {}


def kernel(x, W_ih, W_hh, b_ih, b_hh, w_lin, b_lin, W2, b2, W3, b3):
    x = np.asarray(x, np.float32)
    shared = _prep_shared(np.asarray(W_ih, np.float32), np.asarray(W_hh, np.float32),
                          np.asarray(b_ih, np.float32), np.asarray(b_hh, np.float32),
                          np.asarray(w_lin, np.float32), np.asarray(W2, np.float32),
                          np.asarray(b2, np.float32), np.asarray(W3, np.float32),
                          np.asarray(b3, np.float32))
    b_lin_f = float(np.asarray(b_lin).reshape(-1)[0])

    nc = bacc.Bacc("TRN2", target_bir_lowering=False, debug=False,
                   enable_asserts=False, num_devices=NCORES)
    aps = {}
    aps["xT"] = nc.dram_tensor("xT", (T, A + 1, NL), F32, kind="ExternalInput").ap()
    for name, arr in shared.items():
        aps[name] = nc.dram_tensor(name, arr.shape, F32, kind="ExternalInput").ap()
    aps["out"] = nc.dram_tensor("out", (BL, S), F32, kind="ExternalOutput").ap()

    with tile.TileContext(nc) as tc:
        _build(tc, aps, b_lin_f)
    nc.compile()

    in_maps = []
    for c in range(NCORES):
        xc = x[c * BL:(c + 1) * BL].transpose(1, 3, 0, 2).reshape(T, A, NL)
        xaug = np.concatenate([xc, np.ones((T, 1, NL), np.float32)], axis=1)
        m = {"xT": np.ascontiguousarray(xaug)}
        m.update(shared)
        in_maps.append(m)

    res = bass_utils.run_bass_kernel_spmd(nc, in_maps, core_ids=list(range(NCORES)))
    out = np.concatenate([res.results[c]["out"] for c in range(NCORES)], axis=0)
    return out.astype(np.float32)
